# revision 28
# baseline (speedup 1.0000x reference)
"""Trainium2 Bass kernel for nn_NeuralNet_19516331393457 (dense_mlp).

Pipeline: x = embed[data] (48-entry table); h1 = relu(x@W1+b1);
h2 = tanh(h1@W2+b2); out = h2@W3+b3; return out[argmax(F(out0, out1))].

Strategy (data-parallel over N=500000 on 8 cores; fp8 in/out, bf16 MM2):
  - Host: tiny-table gather embed[data] in fp8e4 (256KB/quad input DMA),
    tile-blocked transpose to [NQ, 128, 2048] per core.
  - Device computes only x->h1->h2 and ships h2 in fp8 (128KB/quad);
    the tiny MM3 (64x2 per sample), F, and argmax run on the host,
    which rescores the top-K=4096 exactly in fp32 anyway (fp8 keeps the
    true winner at rank <=1; the rescore returns the exact answer).
  - Device, software-pipelined per quad q (4 chunks x 512 samples):
      * MM1 x4 (fp8, 1 col/cycle warm) -> 2x [128,1024] PSUM tiles
      * relu evictions split DVE/ACT every quad (PSUM reads are
        1x-rate): relu-a + relu-b hi bank on DVE, relu-b lo bank on
        ACT, so per-quad busy is uniform (ACT ~1.8us, DVE ~2.0us)
      * MM2 x4 (bf16) row-packed 2-up into one [128,1024] PSUM tile; a
        single ACT tanh evicts all 4 chunks to fp8; sync-queue DMA
        ships each h2 tile to HBM
      * PSUM budget: p1 2x2 + p2 2x2 = 8 banks exactly (no MM3/cast
        stage on device, so p2 double-buffers and the MM2->tanh->MM2
        recycling loop never gates)
  - Startup pack: x0 chunk 0 then w1 lead the sync queue (they gate the
    first MM1), w2/biases ride the gpsimd queue, and a tiny tanh on
    memset scratch preloads the ACT function table during the DMA wait.
"""

import numpy as np
import ml_dtypes

import concourse.mybir as mybir
import concourse.tile as tile
from concourse import bacc
from concourse.bass_utils import run_bass_kernel_spmd

N = 500000
D = 128
H1 = 128
H2 = 64
NCLS = 2
NCORES = 8
CHUNK = 512
NPC_RAW = N // NCORES              # 62500 samples per core
NQ = 31                            # quads per core (4 chunks each)
CHUNKS = 4 * NQ                    # 124
NPC = CHUNKS * CHUNK               # 63488 padded samples per core

_F32 = mybir.dt.float32
_BF16 = mybir.dt.bfloat16
_FP8 = mybir.dt.float8e4

NP_FP8 = ml_dtypes.float8_e4m3


def _issue_x_dma(nc, q, pools, tls, xts):
    (xpool, h1pool, h2pool, p1pool, p2pool) = pools
    xt = xpool.tile([D, 4 * CHUNK], _FP8, name=f"xt{q}", tag="xt")
    nc.sync.dma_start(xt[:], tls["x_t"][q, :, :])
    xts[q] = [xt[:, c * CHUNK : (c + 1) * CHUNK] for c in range(4)]


def _quad_head_mm(nc, q, pools, tls, xts):
    """Issue MM1 for quad q (x DMA pre-issued)."""
    (xpool, h1pool, h2pool, p1pool, p2pool) = pools
    xcs = xts.pop(q)

    p1a = p1pool.tile([H1, 2 * CHUNK], _F32, name=f"p1a{q}", tag="p1")
    p1b = p1pool.tile([H1, 2 * CHUNK], _F32, name=f"p1b{q}", tag="p1")
    for h, p1t in ((0, p1a), (1, p1b)):
        for s in range(2):
            nc.tensor.matmul(
                p1t[:, s * CHUNK : (s + 1) * CHUNK],
                tls["w1sb"],
                xcs[2 * h + s],
                start=True, stop=True,
            )
    return p1a, p1b


def _quad_head_relu(nc, q, pools, tls, p1ab):
    (xpool, h1pool, h2pool, p1pool, p2pool) = pools
    p1a, p1b = p1ab
    h1a = h1pool.tile([H1, 2 * CHUNK], _BF16, name=f"h1a{q}", tag="h1")
    nc.vector.tensor_scalar(
        h1a[:], p1a[:], tls["b1sb"], 0.0,
        mybir.AluOpType.add, mybir.AluOpType.max,
    )
    # uniform per-quad split, cut at the PSUM bank boundary: ACT takes
    # relu-b's lo bank (the first h1b block MM2 reads), DVE the hi bank
    h1b = h1pool.tile([H1, 2 * CHUNK], _BF16, name=f"h1b{q}", tag="h1")
    nc.scalar.activation(
        h1b[:, 0:CHUNK], p1b[:, 0:CHUNK],
        mybir.ActivationFunctionType.Relu, bias=tls["b1sb"],
    )
    nc.vector.tensor_scalar(
        h1b[:, CHUNK : 2 * CHUNK], p1b[:, CHUNK : 2 * CHUNK],
        tls["b1sb"], 0.0,
        mybir.AluOpType.add, mybir.AluOpType.max,
    )
    return h1a, h1b


def _quad_tail(nc, q, pools, tls, h1ab):
    """Issue MM2 + tanh->fp8 + h2 out-DMA for quad q."""
    (xpool, h1pool, h2pool, p1pool, p2pool) = pools
    h1a, h1b = h1ab
    p2 = p2pool.tile([128, 2 * CHUNK], _F32, name=f"p2_{q}", tag="p2")
    for h, h1t in ((0, h1a), (1, h1b)):
        for s in range(2):
            nc.tensor.matmul(
                p2[s * H2 : (s + 1) * H2, h * CHUNK : (h + 1) * CHUNK],
                tls["w2sb"],
                h1t[:, s * CHUNK : (s + 1) * CHUNK],
                start=True, stop=True,
            )

    h2t = h2pool.tile([128, 2 * CHUNK], _FP8, name=f"h2_{q}", tag="h2")
    if q == NQ - 1:
        # last quad: tanh + DMA in bank halves (shorter drain tail)
        for h in range(2):
            nc.scalar.activation(
                h2t[:, h * CHUNK : (h + 1) * CHUNK],
                p2[:, h * CHUNK : (h + 1) * CHUNK],
                mybir.ActivationFunctionType.Tanh, bias=tls["b2sb"],
            )
            nc.sync.dma_start(
                tls["h2_d"][q, :, h * CHUNK : (h + 1) * CHUNK],
                h2t[:, h * CHUNK : (h + 1) * CHUNK],
            )
    else:
        nc.scalar.activation(
            h2t[:], p2[:], mybir.ActivationFunctionType.Tanh,
            bias=tls["b2sb"],
        )
        nc.sync.dma_start(tls["h2_d"][q, :, :], h2t[:])


def _build_bass():
    nc = bacc.Bacc(
        "TRN2",
        target_bir_lowering=False,
        debug=False,
        enable_asserts=False,
        num_devices=NCORES,
    )
    x_t = nc.dram_tensor("x_t", [NQ, D, 4 * CHUNK], _FP8,
                         kind="ExternalInput")
    w1pk = nc.dram_tensor("w1pk", [D, H1], _FP8, kind="ExternalInput")
    w2pk = nc.dram_tensor("w2pk", [H1, H2], _BF16, kind="ExternalInput")
    # packed biases: col 0 = b1, col 1 = [b2; b2]
    bpk = nc.dram_tensor("bpk", [128, 2], _F32, kind="ExternalInput")
    # h2 per quad: rows 0:64 = chunk 2h dims, 64:128 = chunk 2h+1 dims,
    # col block h in {0,1}
    h2_d = nc.dram_tensor("h2_d", [NQ, 128, 2 * CHUNK], _FP8,
                          kind="ExternalOutput")

    with tile.TileContext(nc) as tc:
        with (
            tc.tile_pool(name="w", bufs=1) as wpool,
            tc.tile_pool(name="x", bufs=4) as xpool,
            tc.tile_pool(name="x0", bufs=4) as x0pool,
            tc.tile_pool(name="h1", bufs=6) as h1pool,
            tc.tile_pool(name="h2", bufs=4) as h2pool,
            tc.tile_pool(name="p1", bufs=2, space="PSUM") as p1pool,
            tc.tile_pool(name="p2", bufs=2, space="PSUM") as p2pool,
        ):
            tls = {"x_t": x_t, "h2_d": h2_d}
            pools = (xpool, h1pool, h2pool, p1pool, p2pool)
            xts = {}

            # x0 chunk 0 first (it gates the first MM1 together with w1,
            # but its transfer is larger), then w1, then the rest
            x0q = []
            for h in range(4):
                t = x0pool.tile([D, CHUNK], _FP8, name=f"x0_{h}", tag="x0")
                x0q.append(t)
            nc.sync.dma_start(x0q[0][:], x_t[0, :, 0:CHUNK])
            w1sb = wpool.tile([D, H1], _FP8)
            nc.sync.dma_start(w1sb[:], w1pk[:])
            x0r = x0pool.tile([D, 3 * CHUNK], _FP8, name="x0r", tag="x0r")
            nc.sync.dma_start(x0r[:], x_t[0, :, CHUNK : 4 * CHUNK])
            x0q = [x0q[0]] + [x0r[:, h * CHUNK : (h + 1) * CHUNK]
                              for h in range(3)]
            xts[0] = x0q
            _issue_x_dma(nc, 1, pools, tls, xts)
            # scratch memset first on gpsimd (no deps; feeds the ACT
            # table preload and the PE warmup), then w2/biases
            scr = wpool.tile([128, 4], _F32)
            nc.gpsimd.memset(scr[:], 0.0)
            scrw = wpool.tile([128, 256], _F32)
            nc.gpsimd.memset(scrw[:], 0.0)
            bsb = wpool.tile([128, 2], _F32)
            nc.gpsimd.dma_start(bsb[:], bpk[:])
            w2sb = wpool.tile([H1, H2], _BF16)
            nc.gpsimd.dma_start(w2sb[:], w2pk[:])
            tls.update({
                "w1sb": w1sb[:], "w2sb": w2sb[:],
                "b1sb": bsb[:, 0:1], "b2sb": bsb[:, 1:2],
            })

            # preload the ACT function table off the critical path: a tiny
            # tanh on the memset scratch while the first x DMA is in flight
            scr2 = wpool.tile([128, 4], _F32)
            nc.scalar.activation(scr2[:], scr[:],
                                 mybir.ActivationFunctionType.Tanh)

            # warm the PE HAM clock gate during the DMA-wait window: ~3us
            # of tiny dummy matmuls keeps Activity_SHORT busy so the real
            # MM1s start at 2.4GHz instead of 1.2 (flip needs ~3.4us of
            # sustained activity). Distinct psum regions avoid accumulation
            # group churn; the scratch bank is recycled by quad 1's p2.
            pwarm = p2pool.tile([128, 2 * CHUNK], _F32, name="pwarm",
                                tag="p2")
            for j in range(34):
                nc.tensor.matmul(
                    pwarm[0:4, 4 * j : 4 * (j + 1)],
                    scr[:], scr[:],
                    start=True, stop=True,
                )
            # wide fp32 dummies extend the warm-up burst to ~3us of
            # sustained PE activity (the HAM clock-gate needs a full busy
            # window to unthrottle); they end before the x0r-gated fill
            # needs the PE. A single accumulation group into one fixed
            # region keeps the semaphore chain on the recycled p2 banks
            # to a single edge.
            for j in range(9):
                nc.tensor.matmul(
                    pwarm[0:4, 512:768],
                    scrw[:, 0:4], scrw[:],
                    start=(j == 0), stop=(j == 8),
                )

            # interleaved issue so each engine queue matches input-readiness:
            # PE: MM1(q), MM2(q-1); ACT: tanh(q-1), relu-b-lo(q);
            # DVE: relu-a(q), relu-b-hi(q); SYNC: x(q+2), h2(q-1)
            prev_h1 = None
            for q in range(NQ):
                if q + 2 < NQ:
                    _issue_x_dma(nc, q + 2, pools, tls, xts)
                p1ab = _quad_head_mm(nc, q, pools, tls, xts)
                if prev_h1 is not None:
                    _quad_tail(nc, q - 1, pools, tls, prev_h1)
                prev_h1 = _quad_head_relu(nc, q, pools, tls, p1ab)
            _quad_tail(nc, NQ - 1, pools, tls, prev_h1)

    nc.compile()
    return nc


_NC_CACHE = None


def _get_nc():
    global _NC_CACHE
    if _NC_CACHE is None:
        _NC_CACHE = _build_bass()
    return _NC_CACHE


def _F64(x, y):
    return (
        3.0 * (1.0 - x) ** 2 * np.exp(-(x**2) - (y + 1.0) ** 2)
        - 10.0 * (x / 5.0 - x**3 - y**5) * np.exp(-(x**2) - y**2)
        - 1.0 / (3.0 ** np.exp(-((x + 1.0) ** 2) - y**2))
    )


def make_in_maps(data, embed, W1, b1, W2, b2, W3, b3):
    data = np.asarray(data)
    table8 = np.asarray(embed, dtype=np.float32).reshape(-1).astype(NP_FP8)

    w1pk = np.ascontiguousarray(np.asarray(W1, np.float32).astype(NP_FP8))
    w2pk = np.ascontiguousarray(
        np.asarray(W2, np.float32).astype(ml_dtypes.bfloat16))

    b2c = np.asarray(b2, dtype=np.float32).reshape(H2, 1)
    bpk = np.zeros((128, 2), np.float32)
    bpk[:, 0:1] = np.ascontiguousarray(b1, dtype=np.float32).reshape(H1, 1)
    bpk[:, 1:2] = np.concatenate([b2c, b2c], axis=0)

    in_maps = []
    for c in range(NCORES):
        dshard = data[c * NPC_RAW : (c + 1) * NPC_RAW]
        dpad = np.zeros((NPC, D), dtype=dshard.dtype)
        dpad[:NPC_RAW] = dshard
        xt = np.ascontiguousarray(
            table8[dpad.reshape(NQ, 4 * CHUNK, D).transpose(0, 2, 1)]
        )
        in_maps.append({"x_t": xt, "w1pk": w1pk, "w2pk": w2pk, "bpk": bpk})
    return in_maps


def _decode_outs(res, W3, b3):
    """-> out0_all, out1_all fp32 arrays of shape [N] (padding stripped)."""
    W3f = np.asarray(W3, np.float32)
    b3f = np.asarray(b3, np.float32)
    o0s, o1s = [], []
    for c in range(NCORES):
        h2 = np.asarray(res.results[c]["h2_d"], np.float32)
        # [q, r(64-row block), d, h(col block), s] ; chunk = 2h + r
        a = h2.reshape(NQ, 2, 64, 2, CHUNK)
        out = np.einsum('qrdhs,dk->qhrsk', a, W3f, optimize=True) + b3f
        out = out.reshape(NPC, 2)
        o0s.append(out[:NPC_RAW, 0].copy())
        o1s.append(out[:NPC_RAW, 1].copy())
    return np.concatenate(o0s), np.concatenate(o1s)


def kernel(data, embed, W1, b1, W2, b2, W3, b3):
    data = np.asarray(data)
    nc = _get_nc()
    in_maps = make_in_maps(data, embed, W1, b1, W2, b2, W3, b3)
    res = run_bass_kernel_spmd(nc, in_maps, core_ids=list(range(NCORES)))
    o0, o1 = _decode_outs(res, W3, b3)

    pred = _F64(o0.astype(np.float64), o1.astype(np.float64))
    K = 4096
    cand = np.argpartition(pred, N - K)[N - K:]

    table32 = np.asarray(embed, dtype=np.float32).reshape(-1)
    W1f = np.asarray(W1, np.float32)
    W2f = np.asarray(W2, np.float32)
    W3f = np.asarray(W3, np.float32)
    xk = table32[data[cand]]
    hk = np.maximum(xk @ W1f + np.asarray(b1, np.float32), 0.0)
    hk = np.tanh(hk @ W2f + np.asarray(b2, np.float32))
    ok = hk @ W3f + np.asarray(b3, np.float32)
    pk = _F64(ok[:, 0].astype(np.float64), ok[:, 1].astype(np.float64))
    return ok[int(np.argmax(pk))].astype(np.float32)


# revision 29
# speedup vs baseline: 1.0392x; 1.0392x over previous
"""Trainium2 Bass kernel for nn_NeuralNet_19516331393457 (dense_mlp).

Pipeline: x = embed[data] (48-entry table); h1 = relu(x@W1+b1);
h2 = tanh(h1@W2+b2); out = h2@W3+b3; return out[argmax(F(out0, out1))].

Strategy (data-parallel over N=500000 on 8 cores; fp8 in/out, bf16 MM2):
  - Host: tiny-table gather embed[data] in fp8e4 (256KB/quad input DMA),
    tile-blocked transpose to [NQ, 128, 2048] per core.
  - Device computes only x->h1->h2 and ships h2 in fp8 (128KB/quad);
    the tiny MM3 (64x2 per sample), F, and argmax run on the host,
    which rescores the top-K=4096 exactly in fp32 anyway (fp8 keeps the
    true winner at rank <=1; the rescore returns the exact answer).
  - Device, software-pipelined per quad q (4 chunks x 512 samples):
      * MM1 x4 (fp8, 1 col/cycle warm) -> 2x [128,1024] PSUM tiles
      * relu evictions split DVE/ACT every quad (PSUM reads are
        1x-rate): relu-a + relu-b hi bank on DVE, relu-b lo bank on
        ACT, so per-quad busy is uniform (ACT ~1.8us, DVE ~2.0us)
      * MM2 x4 (bf16) row-packed 2-up into one [128,1024] PSUM tile; a
        single ACT tanh evicts all 4 chunks to fp8; sync-queue DMA
        ships each h2 tile to HBM
      * PSUM budget: p1 2x2 + p2 2x2 = 8 banks exactly (no MM3/cast
        stage on device, so p2 double-buffers and the MM2->tanh->MM2
        recycling loop never gates)
  - Startup pack: x0 chunk 0 then w1 lead the sync queue (they gate the
    first MM1), w2/biases ride the gpsimd queue, and a tiny tanh on
    memset scratch preloads the ACT function table during the DMA wait.
"""

import numpy as np
import ml_dtypes

import concourse.mybir as mybir
import concourse.tile as tile
from concourse import bacc
from concourse.bass_utils import run_bass_kernel_spmd

N = 500000
D = 128
H1 = 128
H2 = 64
NCLS = 2
NCORES = 8
CHUNK = 512
NPC_RAW = N // NCORES              # 62500 samples per core
NQ = 31                            # quads per core (4 chunks each)
CHUNKS = 4 * NQ                    # 124
NPC = CHUNKS * CHUNK               # 63488 padded samples per core

_F32 = mybir.dt.float32
_BF16 = mybir.dt.bfloat16
_FP8 = mybir.dt.float8e4

NP_FP8 = ml_dtypes.float8_e4m3


def _issue_x_dma(nc, q, pools, tls, xts):
    (xpool, h1pool, h2pool, p1pool, p2pool) = pools
    xt = xpool.tile([D, 4 * CHUNK], _FP8, name=f"xt{q}", tag="xt")
    nc.sync.dma_start(xt[:], tls["x_t"][q, :, :])
    xts[q] = [xt[:, c * CHUNK : (c + 1) * CHUNK] for c in range(4)]


def _quad_head_mm(nc, q, pools, tls, xts):
    """Issue MM1 for quad q (x DMA pre-issued)."""
    (xpool, h1pool, h2pool, p1pool, p2pool) = pools
    xcs = xts.pop(q)

    p1a = p1pool.tile([H1, 2 * CHUNK], _F32, name=f"p1a{q}", tag="p1")
    p1b = p1pool.tile([H1, 2 * CHUNK], _F32, name=f"p1b{q}", tag="p1")
    for h, p1t in ((0, p1a), (1, p1b)):
        for s in range(2):
            nc.tensor.matmul(
                p1t[:, s * CHUNK : (s + 1) * CHUNK],
                tls["w1sb"],
                xcs[2 * h + s],
                start=True, stop=True,
            )
    return p1a, p1b


def _quad_head_relu(nc, q, pools, tls, p1ab):
    (xpool, h1pool, h2pool, p1pool, p2pool) = pools
    p1a, p1b = p1ab
    h1a = h1pool.tile([H1, 2 * CHUNK], _BF16, name=f"h1a{q}", tag="h1")
    nc.vector.tensor_scalar(
        h1a[:], p1a[:], tls["b1sb"], 0.0,
        mybir.AluOpType.add, mybir.AluOpType.max,
    )
    # uniform per-quad split, cut at the PSUM bank boundary: ACT takes
    # relu-b's lo bank (the first h1b block MM2 reads), DVE the hi bank
    h1b = h1pool.tile([H1, 2 * CHUNK], _BF16, name=f"h1b{q}", tag="h1")
    nc.scalar.activation(
        h1b[:, 0:CHUNK], p1b[:, 0:CHUNK],
        mybir.ActivationFunctionType.Relu, bias=tls["b1sb"],
    )
    nc.vector.tensor_scalar(
        h1b[:, CHUNK : 2 * CHUNK], p1b[:, CHUNK : 2 * CHUNK],
        tls["b1sb"], 0.0,
        mybir.AluOpType.add, mybir.AluOpType.max,
    )
    return h1a, h1b


def _quad_tail(nc, q, pools, tls, h1ab):
    """Issue MM2 + tanh->fp8 + h2 out-DMA for quad q."""
    (xpool, h1pool, h2pool, p1pool, p2pool) = pools
    h1a, h1b = h1ab
    p2 = p2pool.tile([128, 2 * CHUNK], _F32, name=f"p2_{q}", tag="p2")
    for h, h1t in ((0, h1a), (1, h1b)):
        for s in range(2):
            nc.tensor.matmul(
                p2[s * H2 : (s + 1) * H2, h * CHUNK : (h + 1) * CHUNK],
                tls["w2sb"],
                h1t[:, s * CHUNK : (s + 1) * CHUNK],
                start=True, stop=True,
            )

    h2t = h2pool.tile([128, 2 * CHUNK], _FP8, name=f"h2_{q}", tag="h2")
    if q == NQ - 1:
        # last quad: tanh + DMA in bank halves (shorter drain tail)
        for h in range(2):
            nc.scalar.activation(
                h2t[:, h * CHUNK : (h + 1) * CHUNK],
                p2[:, h * CHUNK : (h + 1) * CHUNK],
                mybir.ActivationFunctionType.Tanh, bias=tls["b2sb"],
            )
            nc.sync.dma_start(
                tls["h2_d"][q, :, h * CHUNK : (h + 1) * CHUNK],
                h2t[:, h * CHUNK : (h + 1) * CHUNK],
            )
    else:
        nc.scalar.activation(
            h2t[:], p2[:], mybir.ActivationFunctionType.Tanh,
            bias=tls["b2sb"],
        )
        nc.sync.dma_start(tls["h2_d"][q, :, :], h2t[:])


def _build_bass():
    nc = bacc.Bacc(
        "TRN2",
        target_bir_lowering=False,
        debug=False,
        enable_asserts=False,
        num_devices=NCORES,
    )
    x_t = nc.dram_tensor("x_t", [NQ, D, 4 * CHUNK], _FP8,
                         kind="ExternalInput")
    w1pk = nc.dram_tensor("w1pk", [D, H1], _FP8, kind="ExternalInput")
    w2pk = nc.dram_tensor("w2pk", [H1, H2], _BF16, kind="ExternalInput")
    # packed biases: col 0 = b1, col 1 = [b2; b2]
    bpk = nc.dram_tensor("bpk", [128, 2], _F32, kind="ExternalInput")
    # h2 per quad: rows 0:64 = chunk 2h dims, 64:128 = chunk 2h+1 dims,
    # col block h in {0,1}
    h2_d = nc.dram_tensor("h2_d", [NQ, 128, 2 * CHUNK], _FP8,
                          kind="ExternalOutput")

    with tile.TileContext(nc) as tc:
        with (
            tc.tile_pool(name="w", bufs=1) as wpool,
            tc.tile_pool(name="x", bufs=4) as xpool,
            tc.tile_pool(name="x0", bufs=4) as x0pool,
            tc.tile_pool(name="h1", bufs=6) as h1pool,
            tc.tile_pool(name="h2", bufs=4) as h2pool,
            tc.tile_pool(name="p1", bufs=2, space="PSUM") as p1pool,
            tc.tile_pool(name="p2", bufs=2, space="PSUM") as p2pool,
        ):
            tls = {"x_t": x_t, "h2_d": h2_d}
            pools = (xpool, h1pool, h2pool, p1pool, p2pool)
            xts = {}

            # x0 chunk 0 first (it gates the first MM1 together with w1,
            # but its transfer is larger), then w1, then the rest
            x0q = []
            for h in range(4):
                t = x0pool.tile([D, CHUNK], _FP8, name=f"x0_{h}", tag="x0")
                x0q.append(t)
            nc.sync.dma_start(x0q[0][:], x_t[0, :, 0:CHUNK])
            w1sb = wpool.tile([D, H1], _FP8)
            nc.sync.dma_start(w1sb[:], w1pk[:])
            x0r = x0pool.tile([D, 3 * CHUNK], _FP8, name="x0r", tag="x0r")
            nc.sync.dma_start(x0r[:], x_t[0, :, CHUNK : 4 * CHUNK])
            x0q = [x0q[0]] + [x0r[:, h * CHUNK : (h + 1) * CHUNK]
                              for h in range(3)]
            xts[0] = x0q
            _issue_x_dma(nc, 1, pools, tls, xts)
            # scratch memset first on gpsimd (no deps; feeds the ACT
            # table preload and the PE warmup), then w2/biases
            scr = wpool.tile([128, 4], _F32)
            nc.gpsimd.memset(scr[:], 0.0)
            scrw = wpool.tile([128, 256], _F32)
            nc.gpsimd.memset(scrw[:], 0.0)
            bsb = wpool.tile([128, 2], _F32)
            nc.gpsimd.dma_start(bsb[:], bpk[:])
            w2sb = wpool.tile([H1, H2], _BF16)
            nc.gpsimd.dma_start(w2sb[:], w2pk[:])
            tls.update({
                "w1sb": w1sb[:], "w2sb": w2sb[:],
                "b1sb": bsb[:, 0:1], "b2sb": bsb[:, 1:2],
            })

            # preload the ACT function table off the critical path: a tiny
            # tanh on the memset scratch while the first x DMA is in flight
            scr2 = wpool.tile([128, 4], _F32)
            nc.scalar.activation(scr2[:], scr[:],
                                 mybir.ActivationFunctionType.Tanh)

            # warm the PE HAM clock gate during the DMA-wait window: ~3us
            # of tiny dummy matmuls keeps Activity_SHORT busy so the real
            # MM1s start at 2.4GHz instead of 1.2 (flip needs ~3.4us of
            # sustained activity). Distinct psum regions avoid accumulation
            # group churn; the scratch bank is recycled by quad 1's p2.
            pwarm = p2pool.tile([128, 2 * CHUNK], _F32, name="pwarm",
                                tag="p2")
            for j in range(34):
                nc.tensor.matmul(
                    pwarm[0:4, 4 * j : 4 * (j + 1)],
                    scr[:], scr[:],
                    start=True, stop=True,
                )
            # two wide fp32 dummies (~850ns each at the cold clock) extend
            # the warm-up burst to ~3us of sustained PE activity; they end
            # before the x0c1-gated fill needs the PE, so they cost nothing
            for j in range(2):
                nc.tensor.matmul(
                    pwarm[0:4, 512 + 256 * j : 512 + 256 * (j + 1)],
                    scrw[:, 0:4], scrw[:],
                    start=True, stop=True,
                )

            # interleaved issue so each engine queue matches input-readiness:
            # PE: MM1(q), MM2(q-1); ACT: tanh(q-1), relu-b-lo(q);
            # DVE: relu-a(q), relu-b-hi(q); SYNC: x(q+2), h2(q-1)
            prev_h1 = None
            for q in range(NQ):
                if q + 2 < NQ:
                    _issue_x_dma(nc, q + 2, pools, tls, xts)
                p1ab = _quad_head_mm(nc, q, pools, tls, xts)
                if prev_h1 is not None:
                    _quad_tail(nc, q - 1, pools, tls, prev_h1)
                prev_h1 = _quad_head_relu(nc, q, pools, tls, p1ab)
            _quad_tail(nc, NQ - 1, pools, tls, prev_h1)

    nc.compile()
    return nc


_NC_CACHE = None


def _get_nc():
    global _NC_CACHE
    if _NC_CACHE is None:
        _NC_CACHE = _build_bass()
    return _NC_CACHE


def _F64(x, y):
    return (
        3.0 * (1.0 - x) ** 2 * np.exp(-(x**2) - (y + 1.0) ** 2)
        - 10.0 * (x / 5.0 - x**3 - y**5) * np.exp(-(x**2) - y**2)
        - 1.0 / (3.0 ** np.exp(-((x + 1.0) ** 2) - y**2))
    )


def make_in_maps(data, embed, W1, b1, W2, b2, W3, b3):
    data = np.asarray(data)
    table8 = np.asarray(embed, dtype=np.float32).reshape(-1).astype(NP_FP8)

    w1pk = np.ascontiguousarray(np.asarray(W1, np.float32).astype(NP_FP8))
    w2pk = np.ascontiguousarray(
        np.asarray(W2, np.float32).astype(ml_dtypes.bfloat16))

    b2c = np.asarray(b2, dtype=np.float32).reshape(H2, 1)
    bpk = np.zeros((128, 2), np.float32)
    bpk[:, 0:1] = np.ascontiguousarray(b1, dtype=np.float32).reshape(H1, 1)
    bpk[:, 1:2] = np.concatenate([b2c, b2c], axis=0)

    in_maps = []
    for c in range(NCORES):
        dshard = data[c * NPC_RAW : (c + 1) * NPC_RAW]
        dpad = np.zeros((NPC, D), dtype=dshard.dtype)
        dpad[:NPC_RAW] = dshard
        xt = np.ascontiguousarray(
            table8[dpad.reshape(NQ, 4 * CHUNK, D).transpose(0, 2, 1)]
        )
        in_maps.append({"x_t": xt, "w1pk": w1pk, "w2pk": w2pk, "bpk": bpk})
    return in_maps


def _decode_outs(res, W3, b3):
    """-> out0_all, out1_all fp32 arrays of shape [N] (padding stripped)."""
    W3f = np.asarray(W3, np.float32)
    b3f = np.asarray(b3, np.float32)
    o0s, o1s = [], []
    for c in range(NCORES):
        h2 = np.asarray(res.results[c]["h2_d"], np.float32)
        # [q, r(64-row block), d, h(col block), s] ; chunk = 2h + r
        a = h2.reshape(NQ, 2, 64, 2, CHUNK)
        out = np.einsum('qrdhs,dk->qhrsk', a, W3f, optimize=True) + b3f
        out = out.reshape(NPC, 2)
        o0s.append(out[:NPC_RAW, 0].copy())
        o1s.append(out[:NPC_RAW, 1].copy())
    return np.concatenate(o0s), np.concatenate(o1s)


def kernel(data, embed, W1, b1, W2, b2, W3, b3):
    data = np.asarray(data)
    nc = _get_nc()
    in_maps = make_in_maps(data, embed, W1, b1, W2, b2, W3, b3)
    res = run_bass_kernel_spmd(nc, in_maps, core_ids=list(range(NCORES)))
    o0, o1 = _decode_outs(res, W3, b3)

    pred = _F64(o0.astype(np.float64), o1.astype(np.float64))
    K = 4096
    cand = np.argpartition(pred, N - K)[N - K:]

    table32 = np.asarray(embed, dtype=np.float32).reshape(-1)
    W1f = np.asarray(W1, np.float32)
    W2f = np.asarray(W2, np.float32)
    W3f = np.asarray(W3, np.float32)
    xk = table32[data[cand]]
    hk = np.maximum(xk @ W1f + np.asarray(b1, np.float32), 0.0)
    hk = np.tanh(hk @ W2f + np.asarray(b2, np.float32))
    ok = hk @ W3f + np.asarray(b3, np.float32)
    pk = _F64(ok[:, 0].astype(np.float64), ok[:, 1].astype(np.float64))
    return ok[int(np.argmax(pk))].astype(np.float32)
